# revision 1
# baseline (speedup 1.0000x reference)
"""Trainium2 Bass kernel for AttnPainterOil-style top-K stroke compositing.

Problem semantics (per pixel, fully independent):
  draw[n] = (n+1) * (alpha[n] > 0.1); top-K=10 of draw over N=256 strokes
  (descending) == the 10 highest-index strokes with alpha > 0.1 (for the
  target input distribution every pixel has >= 10 passing strokes, checked
  on the host below).  Gather alpha/color at those indices and composite
  back-to-front over a white canvas.

Streaming formulation used on device (front-to-back, strokes in descending
index order): maintain per-pixel transmittance T (init 1), qualifying-count
cnt (init 0) and color accumulator C (init 0).  For each stroke:
  g   = 1{cnt_before < 10}            (gate; first 10 qualifying win)
  ae  = a * 1{a > 0.1} * g
  cnt += 1{a > 0.1}
  ta  = ae * T ;  T -= ta ;  C += ta * c
Final canvas = C + T (white background).

Only the top D=20 strokes can ever enter any pixel's top-10 (the host
verifies >= 10 passing within the top D per pixel before using the device
path; anything else falls back to an exact host replication).

Engine/dataflow design (v1 all-DVE/f32: ~57us):
  * fp16 end to end on DVE: tensor_tensor runs in the 2x DVE perf mode
    (measured: [128,128] fp16 tt = 134ns vs 200ns f32).
  * ae0 = a*1{a>0.1} resolved on host in f32 (exact threshold), shipped
    fp16, halving input DMA.
  * Count/gate chain off DVE's 1x-stt path: ACT computes q = Sigmoid(
    1000*ae0-50) (exactly 0/1 for ae0 in {0} U (0.1,1)) and gates
    g = Sigmoid(-40*cnt+380); cnt tiles are paired [cnt_odd, cnt_even]
    so ONE ACT op emits both gates of a stroke pair (ACT has a ~370ns
    SBUF bubble per op).  All ACT ops are Sigmoid: a second function-set
    table load (~1.3us stall) never happens.  ACT co-runs with DVE with
    zero interference (measured).
  * Dependent back-to-back DVE ops pay a ~90ns SBUF write-ack penalty;
    independent ops interleave free.  Where the serial T-chain has no
    independent work to lace in (strokes 0-5, 14-19), it switches to a
    sign-alternating fused form X_{i+1} = (ae_i - 1) * X_i (one 1x stt,
    193ns) with the weight ta_i = X_i + X_{i+1} emitted one stroke later
    as the independent filler (y_i = (-1)^i ta_i; the host pre-negates
    odd-stroke colors so PE accumulates the correct sign).  Strokes 6-13
    stay in plain mult/sub form, fully laced with the independent cnt
    adds, the cnt_9 pairwise-tree, and the gate multiplies.
  * PE accumulates weighted colors into PSUM via fp16 identity matmuls.
    Dummy keepalive matmuls hold PE utilization up: HAM only grants the
    full clock (0.96 GHz DVE / 2.4 GHz PE vs 0.8 / 2.0) under sustained
    PE activity, measured 35.4us -> 30.6us from this alone.
  * All input DMAs dispatched up front, need-ordered (each SP dma_start
    is ~600ns serial dispatch + ~1.8us queue latency).

Sharding: pure data parallel, one batch element per NeuronCore (B=8).
"""

import numpy as np

B, N, W, K = 8, 256, 128, 10
ALPHA_THRESH = 0.1
D = 20          # strokes processed from the top (must cover every pixel's
                # top-10; exact minimum for the fixed key=0 input — verified,
                # and kernel() checks the precondition before the device path)
P = 128         # partitions (pixel rows)
F = 128         # free dim (pixel cols)
NCORES = 8

# gate = Sigmoid(GATE_SCALE*cnt + GATE_BIAS): cnt<=9 -> 1.0, cnt>=10 -> 0.0 (fp16)
GATE_SCALE = -40.0
GATE_BIAS = 9.5 * 40.0

# strokes run in the sign-alternating stt form (see module docstring);
# the host negates the colors of the odd ones
STT_STROKES = (0, 1, 2, 3, 4, 5, 14, 15, 16, 17, 18, 19)
NEG_STROKES = (1, 3, 5, 15, 17, 19)

_nc_cache = {}


def _build_nc(depth):
    import concourse.bass as bass  # noqa: F401
    import concourse.tile as tile
    from concourse import bacc, mybir
    from concourse.vector_clock import ScopedClock

    op = mybir.AluOpType
    f32 = mybir.dt.float32
    f16 = mybir.dt.float16
    actf = mybir.ActivationFunctionType
    assert depth == 20, "emission schedule below is specialized for D=20"

    class _OneShotTileContext(tile.TileContext):
        """TileContext with a slim exit: the drain alone (it waits on the
        global clock, including output-DMA completion) — no all-engine
        barriers and no per-semaphore clears.  Safe because every
        run_bass_kernel_spmd call builds and loads a fresh executable, so
        semaphore state never carries across runs."""

        def _drain_and_barrier(self, tick_clock, wait_clock):
            drain_inst = self.nc.sync.drain()
            wait_clock.add_sem_waits(
                drain_inst.ins, ScopedClock({None: tick_clock.global_clock})
            )
            popped = self.nc._tile_sem_poison_stack.pop()
            assert popped is self._sem_poison

    nc = bacc.Bacc("TRN2", target_bir_lowering=False, debug=False)

    ae_d = nc.dram_tensor("ae_in", [P, depth * F], f16, kind="ExternalInput").ap()
    color_d = nc.dram_tensor("color_in", [P, depth * 3 * F], f16, kind="ExternalInput").ap()
    ident_d = nc.dram_tensor("ident_in", [P, P], f16, kind="ExternalInput").ap()
    out_d = nc.dram_tensor("out", [P, 3 * F], f16, kind="ExternalOutput").ap()

    ae_regions = [(0, 2), (2, 6), (6, depth)]

    with _OneShotTileContext(nc) as tc:
        with (
            tc.tile_pool(name="const", bufs=1) as constp,
            tc.tile_pool(name="state", bufs=1) as statep,
            tc.tile_pool(name="x", bufs=4) as xp,
            tc.tile_pool(name="cnt", bufs=5) as cntp,
            tc.tile_pool(name="gate", bufs=4) as gatep,
            tc.tile_pool(name="aeg", bufs=4) as aegp,
            tc.tile_pool(name="cchunk", bufs=4) as cchunkp,
            tc.tile_pool(name="tap", bufs=4) as tap,
            tc.tile_pool(name="prodp", bufs=4) as prodp,
            tc.tile_pool(name="psum", bufs=1, space="PSUM") as psump,
        ):
            # --- constants / state (all off the DVE critical path) ---
            ident = constp.tile([P, P], f16)
            X0 = statep.tile([P, F], f16)       # transmittance chain head
            cnt0 = statep.tile([P, F], f16)
            warm = statep.tile([P, 1], f16)
            gbias = statep.tile([P, 1], f32)
            qbias = statep.tile([P, 1], f32)
            fdum = statep.tile([P, 1], f16)     # DVE ack-latency filler
            # force the ACT Sigmoid-table load at t~0 (it otherwise stalls
            # the first real ACT op by ~1.3us); every ACT op in this kernel
            # is a Sigmoid so the table never reloads
            nc.scalar.activation(warm[:], warm[:], func=actf.Sigmoid,
                                 bias=gbias[:], scale=GATE_SCALE)

            cacc = psump.tile([P, 3 * F], f32)
            scratch = psump.tile([P, 3 * F], f32)

            # PE warmup off the gpsimd-memset cnt0 tile (lands ~7.3us, never
            # rewritten): HAM clock ramp completes before real compute
            for _ in range(14):
                nc.tensor.matmul(
                    scratch[:, :F], cnt0[:], cnt0[:],
                    start=True, stop=True, skip_group_check=True,
                )

            def pe_keepalive(n):
                # PE is ~70% idle; HAM only grants the full clock under
                # sustained PE utilization.  Free: off the critical path.
                for _ in range(n):
                    nc.tensor.matmul(
                        scratch[:], ident[:],
                        ae_t[:, : 3 * F], start=True, stop=True,
                        skip_group_check=True,
                    )

            # --- all input DMAs dispatched up front, need-ordered ---
            # the two opening transfers ride the SWDGE (gpsimd) queue: its
            # sequencer is past the preamble ~1.3us before SP, so the first
            # compute starts that much earlier
            ae_t = statep.tile([P, depth * F], f16)
            q_t = statep.tile([P, depth * F], f16)

            def dma_ae(ri, eng):
                lo, hi = ae_regions[ri]
                eng.dma_start(
                    ae_t[:, lo * F : hi * F], ae_d[:, lo * F : hi * F]
                )

            cchunks = {}

            def dma_cchunk(lo, hi, eng):
                cchunk = cchunkp.tile([P, 8, 3, F], f16, tag="cchunk", name="cchunk")
                eng.dma_start(
                    cchunk[:, : hi - lo],
                    color_d[:, lo * 3 * F : hi * 3 * F].rearrange(
                        "p (s c f) -> p s c f", s=hi - lo, c=3
                    ),
                )
                cchunks[lo] = cchunk

            nc.gpsimd.memset(warm[:], 0.0)
            nc.gpsimd.memset(gbias[:], GATE_BIAS)
            nc.gpsimd.memset(X0[:], 1.0)
            nc.gpsimd.memset(cnt0[:], 0.0)
            nc.gpsimd.memset(qbias[:], -50.0)
            nc.gpsimd.dma_start(ident[:], ident_d)
            dma_ae(0, nc.sync)
            dma_cchunk(0, 2, nc.sync)
            dma_ae(1, nc.sync)
            dma_cchunk(2, 6, nc.sync)
            dma_ae(2, nc.sync)
            for lo in range(6, depth, 8):
                dma_cchunk(lo, min(lo + 8, depth), nc.sync)

            # q = 1{ae0 > 0} per region on ACT: ae0 is either 0 or > 0.1,
            # so Sigmoid(1000*ae0 - 50) is exactly 0.0 / 1.0 in fp16
            for lo, hi in ae_regions:
                nc.scalar.activation(
                    q_t[:, lo * F : hi * F], ae_t[:, lo * F : hi * F],
                    func=actf.Sigmoid, bias=qbias[:], scale=1000.0,
                )

            def ae_plane(s, n=1):
                return ae_t[:, s * F : (s + n) * F]

            def q_plane(s):
                return q_t[:, s * F : (s + 1) * F]

            def c_group(s, n):
                if s < 2:
                    lo = 0
                elif s < 6:
                    lo = 2
                else:
                    lo = 6 + ((s - 6) // 8) * 8
                return cchunks[lo][:, s - lo : s - lo + n]

            # cnt pair tile pi holds [cnt_{2pi-1}, cnt_{2pi}] so one ACT op
            # emits both gates of stroke pair (2pi, 2pi+1)
            cnt_tiles = {}
            gate_tiles = {}

            def cnt_slot(t):
                pi = (t + 1) // 2
                return pi, 0 if t % 2 else 1

            def cnt_ap(t):
                if t == -1:
                    return cnt0[:]
                pi, sl = cnt_slot(t)
                return cnt_tiles[pi][:, sl]

            def cnt_dst(t):
                pi, sl = cnt_slot(t)
                if pi not in cnt_tiles:
                    ct = cntp.tile([P, 2, F], f16, tag="cnt", name="cnt")
                    cnt_tiles[pi] = ct
                return cnt_tiles[pi][:, sl]

            def cnt_add(t):
                nc.vector.tensor_tensor(
                    cnt_dst(t), cnt_ap(t - 1), q_plane(t), op=op.add
                )
                if t % 2 == 0 and t >= K and t <= depth - 2:
                    pi = t // 2
                    gtile = gatep.tile([P, 2, F], f16, tag="gate", name="gate")
                    gate_tiles[t] = gtile
                    nc.scalar.activation(
                        gtile[:].rearrange("p s f -> p (s f)"),
                        cnt_tiles[pi][:].rearrange("p s f -> p (s f)"),
                        func=actf.Sigmoid, bias=gbias[:], scale=GATE_SCALE,
                    )

            def filler():
                nc.vector.memset(fdum[:], 0.0)

            def aeg_pair(u):
                # gated effective alphas for stroke pair (u, u+1)
                t = aegp.tile([P, 2, F], f16, tag="aeg", name="aeg")
                nc.vector.tensor_tensor(
                    t[:].rearrange("p s f -> p (s f)"), ae_plane(u, 2),
                    gate_tiles[u][:].rearrange("p s f -> p (s f)"), op=op.mult,
                )
                return t

            def new_x():
                return xp.tile([P, F], f16, tag="x", name="xt")

            def stt_step(x_prev, ae_ap):
                # X_{i+1} = (ae_i - 1) * X_i   (1x stt; sign alternates)
                x_nxt = new_x()
                nc.vector.scalar_tensor_tensor(
                    x_nxt[:], ae_ap, 1.0, x_prev[:], op0=op.subtract, op1=op.mult
                )
                return x_nxt

            def y_op(dst, xa, xb):
                # y_i = X_i + X_{i+1} = (-1)^i ta_i
                nc.vector.tensor_tensor(dst, xa[:], xb[:], op=op.add)

            def prod_group(ta_ap, s, n, eng=None):
                pr = prodp.tile([P, 4, 3, F], f16, tag="prod", name="prod")
                ta_b = ta_ap.unsqueeze(2).broadcast_to((P, n, 3, F))
                (eng or nc.vector).tensor_tensor(
                    pr[:, :n], c_group(s, n), ta_b, op=op.mult
                )
                return pr

            def matmuls(pr, s, n, ka=3, stop_last=False):
                # ka=0 near the end: late keepalives would sit between the
                # last real matmuls and the closing write, delaying the
                # output by ~1us.  stop_last marks the group's final write.
                for j in range(n):
                    nc.tensor.matmul(
                        cacc[:], ident[:],
                        pr[:, j].rearrange("p c f -> p (c f)"),
                        start=(s + j == 0),
                        stop=(stop_last and j == n - 1),
                        skip_group_check=True,
                    )
                pe_keepalive(ka)

            # ---------------- emission schedule (D=20) ----------------
            # Phase AB: strokes 0-5, stt form (self-interleaving: the y ops
            # are the independent fillers between the serial stt steps)
            tg01 = tap.tile([P, 4, F], f16, tag="ta")
            tg23 = tap.tile([P, 4, F], f16, tag="ta")
            tg45 = tap.tile([P, 4, F], f16, tag="ta")
            X1 = stt_step(X0, ae_plane(0))
            filler()
            X2 = stt_step(X1, ae_plane(1))
            y_op(tg01[:, 0], X0, X1)
            X3 = stt_step(X2, ae_plane(2))
            y_op(tg01[:, 1], X1, X2)
            X4 = stt_step(X3, ae_plane(3))
            y_op(tg23[:, 0], X2, X3)
            p01 = prod_group(tg01[:, :2], 0, 2)
            X5 = stt_step(X4, ae_plane(4))
            y_op(tg23[:, 1], X3, X4)
            X6 = stt_step(X5, ae_plane(5))
            y_op(tg45[:, 0], X4, X5)
            p23 = prod_group(tg23[:, :2], 2, 2)
            filler()
            y_op(tg45[:, 1], X5, X6)
            matmuls(p01, 0, 2)
            filler()
            p45 = prod_group(tg45[:, :2], 4, 2)
            matmuls(p23, 2, 2)
            matmuls(p45, 4, 2)

            # Phase C: strokes 6-9 mult form (T lives in-place in X6's tile);
            # the cnt_9 tree (cnt_0..8 are never read) + cnt 10-12 + aeg10
            # lace the chain gaps
            T = X6
            qv = q_t[:, : 10 * F].rearrange("p (s two f) -> p s two f", two=2, f=F)
            t5 = statep.tile([P, 5, F], f16)
            t2 = statep.tile([P, 2, F], f16)
            t1 = statep.tile([P, F], f16)

            def ta_sub(tg_ap, ae_ap):
                nc.vector.tensor_tensor(tg_ap, ae_ap, T[:], op=op.mult)

            def T_sub(tg_ap):
                nc.vector.tensor_tensor(T[:], T[:], tg_ap, op=op.subtract)

            tg69 = tap.tile([P, 4, F], f16, tag="ta")
            ta_sub(tg69[:, 0], ae_plane(6))
            nc.vector.tensor_tensor(t5[:], qv[:, :, 0], qv[:, :, 1], op=op.add)
            T_sub(tg69[:, 0])
            nc.vector.tensor_tensor(t2[:], t5[:, 0:2], t5[:, 2:4], op=op.add)
            ta_sub(tg69[:, 1], ae_plane(7))
            nc.vector.tensor_tensor(t1[:], t2[:, 0], t2[:, 1], op=op.add)
            T_sub(tg69[:, 1])
            nc.vector.tensor_tensor(cnt_dst(9), t1[:], t5[:, 4], op=op.add)
            ta_sub(tg69[:, 2], ae_plane(8))
            cnt_add(10)
            T_sub(tg69[:, 2])
            cnt_add(11)
            ta_sub(tg69[:, 3], ae_plane(9))
            cnt_add(12)
            T_sub(tg69[:, 3])
            aeg10 = aeg_pair(10)
            p69 = prod_group(tg69[:], 6, 4)
            matmuls(p69, 6, 4)

            # Phase D: strokes 10-13 (gated); cnt 13-18 + aeg12/14 laced in
            tg1013 = tap.tile([P, 4, F], f16, tag="ta")
            ta_sub(tg1013[:, 0], aeg10[:, 0])
            cnt_add(13)
            T_sub(tg1013[:, 0])
            cnt_add(14)
            ta_sub(tg1013[:, 1], aeg10[:, 1])
            cnt_add(15)
            T_sub(tg1013[:, 1])
            aeg12 = aeg_pair(12)
            cnt_add(16)
            ta_sub(tg1013[:, 2], aeg12[:, 0])
            cnt_add(17)
            T_sub(tg1013[:, 2])
            cnt_add(18)
            ta_sub(tg1013[:, 3], aeg12[:, 1])
            filler()
            T_sub(tg1013[:, 3])
            aeg14 = aeg_pair(14)
            p1013 = prod_group(tg1013[:], 10, 4)
            matmuls(p1013, 10, 4)

            # Phase E: strokes 14-19, stt form (gated); y ops fill the gaps.
            # Products go to PE in pairs (not a quad) so the PSUM stream
            # finishes earlier: the final add waits on PE's last cacc write.
            tgE = tap.tile([P, 4, F], f16, tag="ta")
            tgF = tap.tile([P, 4, F], f16, tag="ta")
            X15 = stt_step(T, aeg14[:, 0])
            aeg16 = aeg_pair(16)
            X16 = stt_step(X15, aeg14[:, 1])
            y_op(tgE[:, 0], T, X15)
            X17 = stt_step(X16, aeg16[:, 0])
            y_op(tgE[:, 1], X15, X16)
            aeg18 = aeg_pair(18)
            X18 = stt_step(X17, aeg16[:, 1])
            y_op(tgE[:, 2], X16, X17)
            p1415 = prod_group(tgE[:, :2], 14, 2)
            matmuls(p1415, 14, 2, ka=0)
            X19 = stt_step(X18, aeg18[:, 0])
            y_op(tgE[:, 3], X17, X18)
            # the last four strokes' products go as singles right after each
            # weight lands: PE's matmuls and the DVE tail sum each start as
            # early as their data exists instead of behind a pair barrier
            p16 = prod_group(tgE[:, 2:3], 16, 1)
            matmuls(p16, 16, 1, ka=0)
            X20 = stt_step(X19, aeg18[:, 1])
            y_op(tgF[:, 0], X18, X19)
            p17 = prod_group(tgE[:, 3:4], 17, 1)
            matmuls(p17, 17, 1, ka=0)
            # white background: T_final folded into PSUM by PE (one matmul
            # with X20 broadcast across the 3 channel blocks).  Emitted
            # BEFORE mm18: X20 is ready early, so PE runs it while DVE is
            # still producing p18, and mm18 becomes the closing cacc write.
            nc.tensor.matmul(
                cacc[:].rearrange("p (c f) -> p c f", c=3), ident[:],
                X20[:].unsqueeze(1).broadcast_to((P, 3, F)),
                start=False, stop=False, skip_group_check=True,
            )
            y_op(tgF[:, 1], X19, X20)
            p18 = prod_group(tgF[:, 0:1], 18, 1)
            matmuls(p18, 18, 1, ka=0, stop_last=True)
            filler()
            p19 = prod_group(tgF[:, 1:2], 19, 1)

            # tail: canvas = C_psum(+T+prod18) + prod19; only stroke 19's
            # product stays on DVE, so the tail is a single add
            out_t = constp.tile([P, 3, F], f16, tag="out")
            nc.vector.tensor_tensor(
                out_t[:], cacc[:].rearrange("p (c f) -> p c f", c=3), p19[:, 0],
                op=op.add,
            )
            nc.sync.dma_start(out_d, out_t[:].rearrange("p c f -> p (c f)"))

    nc.compile()
    return nc


def _prep_inputs(color_stroke, alpha, depth):
    """Slice the top `depth` strokes (reversed so stroke 0 = highest index),
    resolve the alpha threshold in f32 on host, and lay out per core in fp16:
    ae [P, depth*F], color [P, depth*3*F].  Colors of NEG_STROKES are
    negated: those strokes' weights come out of the sign-alternating stt
    chain as -ta (see _build_nc)."""
    a_r = alpha[:, N - depth :, 0][:, ::-1]          # (B, depth, P, F) f32
    ae0 = (a_r * (a_r > ALPHA_THRESH)).astype(np.float16)
    c_r = color_stroke[:, N - depth :][:, ::-1].astype(np.float16)  # (B, depth, 3, P, F)
    c_r = c_r.copy()
    c_r[:, list(NEG_STROKES)] = -c_r[:, list(NEG_STROKES)]
    ident = np.eye(P, dtype=np.float16)
    in_maps = []
    for b in range(B):
        a_core = np.ascontiguousarray(ae0[b].transpose(1, 0, 2)).reshape(P, depth * F)
        c_core = np.ascontiguousarray(c_r[b].transpose(2, 0, 1, 3)).reshape(
            P, depth * 3 * F
        )
        in_maps.append(
            {"ae_in": a_core, "color_in": c_core, "ident_in": ident}
        )
    return in_maps


def _reference_numpy(color_stroke, alpha):
    """Exact replication of the oracle (incl. top-k tie-breaking) on host.
    Only used when the depth-cutoff precondition fails (pathological inputs)."""
    stroke_ids = np.arange(1, N + 1, dtype=np.int32).reshape(1, N, 1, 1)
    draw = stroke_ids * (alpha[:, :, 0] > ALPHA_THRESH).astype(np.int32)  # (B,N,W,W)
    draw_t = np.moveaxis(draw, 1, -1)  # (B,W,W,N)
    idx = np.argsort(-draw_t, axis=-1, kind="stable")[..., :K]  # (B,W,W,K)
    idx = np.moveaxis(idx, -1, 1)[:, :, None]  # (B,K,1,W,W)
    alpha_k = np.take_along_axis(alpha, idx, axis=1)  # (B,K,1,W,W)
    color_k = np.take_along_axis(color_stroke, idx, axis=1)  # (B,K,3,W,W)
    canvas = np.ones((B, 3, W, W), dtype=color_stroke.dtype)
    for i in range(K - 1, -1, -1):
        a = alpha_k[:, i]
        canvas = canvas * (1.0 - a) + a * color_k[:, i]
    return canvas


def kernel(color_stroke, alpha):
    color_stroke = np.asarray(color_stroke, dtype=np.float32)
    alpha = np.asarray(alpha, dtype=np.float32)
    assert color_stroke.shape == (B, N, 3, W, W), color_stroke.shape
    assert alpha.shape == (B, N, 1, W, W), alpha.shape

    # Precondition for the depth cutoff: every pixel finds its 10 passing
    # strokes within the top D.
    top_pass = (alpha[:, N - D :, 0] > ALPHA_THRESH).sum(axis=1)
    if top_pass.min() < K:
        return _reference_numpy(color_stroke, alpha)

    from concourse.bass_utils import run_bass_kernel_spmd

    if D not in _nc_cache:
        _nc_cache[D] = _build_nc(D)
    nc = _nc_cache[D]

    in_maps = _prep_inputs(color_stroke, alpha, D)
    res = run_bass_kernel_spmd(nc, in_maps, core_ids=list(range(NCORES)))

    out = np.empty((B, 3, W, W), dtype=np.float32)
    for b in range(B):
        out[b] = (
            res.results[b]["out"].astype(np.float32).reshape(P, 3, F).transpose(1, 0, 2)
        )
    return out



# revision 3
# speedup vs baseline: 1.1121x; 1.1121x over previous
"""Trainium2 Bass kernel for AttnPainterOil-style top-K stroke compositing.

Problem semantics (per pixel, fully independent):
  draw[n] = (n+1) * (alpha[n] > 0.1); top-K=10 of draw over N=256 strokes
  (descending) == the 10 highest-index strokes with alpha > 0.1.  Gather
  alpha/color at those indices and composite back-to-front over a white
  canvas.  Only the top D=20 strokes can enter any pixel's top-10 (host
  verifies the precondition; exact host fallback otherwise).

v2 formulation (dc-telescoping): with T_i the transmittance before stroke i
(T_0 = 1, T_{i+1} = T_i * (1 - aeg_i), aeg = gated effective alpha) the
composite

  canvas = sum_i (T_i - T_{i+1}) c_i + T_20
         = T_0 c_0 + sum_{i=1..19} T_i (c_i - c_{i-1}) + T_20 (1 - c_19)

so with HOST-precomputed color differences dc_i the device never extracts
per-stroke weights ta_i = T_i - T_{i+1}: products use the T planes directly.
The device chain runs sign-alternating (X_i = (-1)^i T_i) so each gated step
is ONE scalar_tensor_tensor (aeg-1)*X; the host bakes the (-1)^i into dc.

Device work:
  * top-k selection: q = 1{alpha>0.1} (ACT sigmoid trick), qualifying-count
    tree + pair chain (DVE), gates g = 1{cnt<=9} (ACT), aeg = ae*g (DVE).
  * chain: strokes 0-9 advance in PAIRS X_{2p+2} = X_2p * M_p with
    M_p = am_e*am_o (am = ae-1, host-shipped for the ungated strokes); odd
    planes X_{2p+1} = X_2p * am_e land in ONE strided 5F op.  Strokes 10-19
    advance per stroke via stt (aeg-1)*X (absorbs the -1 for free).
  * products: pr = X (bcast over 3 channels) * dc in big batched ops
    (measured 0.57ns/el with broadcast-middle APs); PE accumulates each
    stroke plane into PSUM via fp16 identity matmuls (start: c_0 directly
    from HBM tile; end: X_20 * (1-c_19) plane).
  * tail: ACT Copy converts PSUM f32 -> SBUF f16 (off the DVE), DMA out.

Engine notes kept from v1: ACT only ever runs Sigmoid/Copy (one table set,
single ~1.3us load at t~0), PE warmup + keepalive matmuls hold the HAM
clock up, all input DMAs dispatched up front need-ordered.

Sharding: pure data parallel, one batch element per NeuronCore (B=8).
"""

import numpy as np

B, N, W, K = 8, 256, 128, 10
ALPHA_THRESH = 0.1
D = 20          # strokes processed from the top (covers every pixel's
                # top-10 for the target inputs; checked before device path)
P = 128         # partitions (pixel rows)
F = 128         # free dim (pixel cols)
NCORES = 8

# gate = Sigmoid(GATE_SCALE*cnt + GATE_BIAS): cnt<=9 -> 1.0, cnt>=10 -> 0.0
GATE_SCALE = -40.0
GATE_BIAS = 9.5 * 40.0

_nc_cache = {}


def _build_nc(depth):
    import concourse.bass as bass  # noqa: F401
    import concourse.tile as tile
    from concourse import bacc, mybir
    from concourse.vector_clock import ScopedClock

    op = mybir.AluOpType
    f32 = mybir.dt.float32
    f16 = mybir.dt.float16
    actf = mybir.ActivationFunctionType
    assert depth == 20, "emission schedule below is specialized for D=20"

    class _OneShotTileContext(tile.TileContext):
        """TileContext with a slim exit: the drain alone (it waits on the
        global clock, including output-DMA completion) — no all-engine
        barriers and no per-semaphore clears.  Safe because every
        run_bass_kernel_spmd call builds and loads a fresh executable, so
        semaphore state never carries across runs."""

        def _drain_and_barrier(self, tick_clock, wait_clock):
            drain_inst = self.nc.sync.drain()
            wait_clock.add_sem_waits(
                drain_inst.ins, ScopedClock({None: tick_clock.global_clock})
            )
            popped = self.nc._tile_sem_poison_stack.pop()
            assert popped is self._sem_poison

    nc = bacc.Bacc("TRN2", target_bir_lowering=False, debug=False)

    # aeh planes 0-9: am = ae-1 (ungated strokes); planes 10-19: raw ae
    aeh_d = nc.dram_tensor("aeh_in", [P, depth * F], f16, kind="ExternalInput").ap()
    # dc planes: dc_0 = c_0; dcS_i = (-1)^i (c_i - c_{i-1}); dc_20 = 1 - c_19
    dc_d = nc.dram_tensor("dc_in", [P, (depth + 1) * 3 * F], f16,
                          kind="ExternalInput").ap()
    ident_d = nc.dram_tensor("ident_in", [P, P], f16, kind="ExternalInput").ap()
    out_d = nc.dram_tensor("out", [P, 3 * F], f16, kind="ExternalOutput").ap()

    with _OneShotTileContext(nc) as tc:
        with (
            tc.tile_pool(name="const", bufs=1) as constp,
            tc.tile_pool(name="state", bufs=1) as statep,
            tc.tile_pool(name="cntq", bufs=1) as cntqp,
            tc.tile_pool(name="gate", bufs=6) as gatep,
            tc.tile_pool(name="aeg", bufs=6) as aegp,
            tc.tile_pool(name="prod", bufs=4) as prodp,
            tc.tile_pool(name="psum", bufs=1, space="PSUM") as psump,
        ):
            # --- constants / state (all off the DVE critical path) ---
            ident = constp.tile([P, P], f16)
            aeh = statep.tile([P, depth, F], f16)
            dc = statep.tile([P, depth + 1, 3, F], f16)
            X = statep.tile([P, depth + 1, F], f16)   # X_i = (-1)^i T_i
            M04 = statep.tile([P, 5, F], f16)
            qA = statep.tile([P, 10, F], f16)
            qB = statep.tile([P, 9, F], f16)
            qp = statep.tile([P, 4, F], f16)
            s5 = statep.tile([P, 5, F], f16)
            s2 = statep.tile([P, 2, F], f16)
            s1 = statep.tile([P, F], f16)
            warm = statep.tile([P, 1], f16)
            gbias = statep.tile([P, 1], f32)
            qbiasA = statep.tile([P, 1], f32)   # +950 (q from am planes)
            qbiasB = statep.tile([P, 1], f32)   # -50  (q from ae planes)
            zero4 = statep.tile([P, 1], f16)    # PE warmup rhs
            # force the ACT Sigmoid-table load at t~0; every ACT op here is
            # Sigmoid or Copy (copy lives in every table set: no reload)
            nc.scalar.activation(warm[:], warm[:], func=actf.Sigmoid,
                                 bias=gbias[:], scale=GATE_SCALE)

            cacc = psump.tile([P, 3 * F], f32)
            scratch = psump.tile([P, 3 * F], f32)

            # PE warmup off the gpsimd-memset zero tile: HAM clock ramp
            # completes before real compute
            for _ in range(14):
                nc.tensor.matmul(
                    scratch[:, :F], zero4[:].broadcast_to((P, F)),
                    zero4[:].broadcast_to((P, F)),
                    start=True, stop=True, skip_group_check=True,
                )

            def pe_keepalive(n):
                for _ in range(n):
                    nc.tensor.matmul(
                        scratch[:], ident[:],
                        aeh[:, 0:3].rearrange("p s f -> p (s f)"),
                        start=True, stop=True, skip_group_check=True,
                    )

            # --- memsets + all input DMAs, need-ordered ---
            nc.gpsimd.memset(warm[:], 0.0)
            nc.gpsimd.memset(gbias[:], GATE_BIAS)
            nc.gpsimd.memset(qbiasA[:], 950.0)
            nc.gpsimd.memset(qbiasB[:], -50.0)
            nc.gpsimd.memset(zero4[:], 0.0)
            nc.gpsimd.memset(X[:, 0], 1.0)          # X_0 = T_0 = 1
            # sync (HWDGE) queue: aeh halves then dc chunks in consumption
            # order; gpsimd (SWDGE) queue: ident (PE needs it ~t+4.5us)
            nc.sync.dma_start(aeh[:, 0:10], aeh_d[:, : 10 * F].rearrange(
                "p (s f) -> p s f", f=F))
            nc.sync.dma_start(aeh[:, 10:20], aeh_d[:, 10 * F:].rearrange(
                "p (s f) -> p s f", f=F))
            nc.gpsimd.dma_start(ident[:], ident_d)
            dc_ranges = [(0, 6), (6, 11), (11, 16), (16, 21)]
            for lo, hi in dc_ranges:
                nc.sync.dma_start(
                    dc[:, lo:hi],
                    dc_d[:, lo * 3 * F: hi * 3 * F].rearrange(
                        "p (s c f) -> p s c f", c=3, f=F),
                )

            # --- ACT stream: q planes, then gates as cnt pairs complete ---
            # q = 1{alpha > 0.1}: planes 0-9 hold am = ae-1 (ae>0 <=> am>-1)
            # so Sigmoid(1000*am + 950) is exactly 0/1; planes 10-18 hold ae.
            nc.scalar.activation(
                qA[:].rearrange("p s f -> p (s f)"),
                aeh[:, 0:10].rearrange("p s f -> p (s f)"),
                func=actf.Sigmoid, bias=qbiasA[:], scale=1000.0,
            )
            nc.scalar.activation(
                qB[:].rearrange("p s f -> p (s f)"),
                aeh[:, 10:19].rearrange("p s f -> p (s f)"),
                func=actf.Sigmoid, bias=qbiasB[:], scale=1000.0,
            )

            # cnt pair tile for pair p (strokes 2p, 2p+1): [cnt_{2p-1}, cnt_{2p}]
            cntp = {p: cntqp.tile([P, 2, F], f16, name=f"cntp{p}")
                    for p in range(5, 10)}
            gtiles = {}
            aegs = {}

            def gate(p):
                g = gatep.tile([P, 2, F], f16, tag="gate", name="gate")
                gtiles[p] = g
                nc.scalar.activation(
                    g[:].rearrange("p s f -> p (s f)"),
                    cntp[p][:].rearrange("p s f -> p (s f)"),
                    func=actf.Sigmoid, bias=gbias[:], scale=GATE_SCALE,
                )

            def aeg(p):
                t = aegp.tile([P, 2, F], f16, tag="aeg", name="aeg")
                aegs[p] = t
                nc.vector.tensor_tensor(
                    t[:].rearrange("p s f -> p (s f)"),
                    aeh[:, 2 * p: 2 * p + 2].rearrange("p s f -> p (s f)"),
                    gtiles[p][:].rearrange("p s f -> p (s f)"), op=op.mult,
                )

            aeh_pairsA = aeh[:, 0:10].rearrange("p (s two) f -> p s two f", two=2)
            am_even = aeh_pairsA[:, :, 0]          # planes 0,2,4,6,8
            am_odd = aeh_pairsA[:, :, 1]           # planes 1,3,5,7,9
            X_evenA = X[:, 0:10].rearrange("p (s two) f -> p s two f", two=2)[:, :, 0]
            X_oddA = X[:, 1:11].rearrange("p (s two) f -> p s two f", two=2)[:, :, 0]

            def chainA(p):
                # X_{2p+2} = X_{2p} * M_p   (all non-negative: pair factors)
                nc.vector.tensor_tensor(
                    X[:, 2 * p + 2], X[:, 2 * p], M04[:, p], op=op.mult)

            def sttB(i):
                # X_{i+1} = (aeg_i - 1) * X_i
                pa = aegs[i // 2][:, i % 2]
                nc.vector.scalar_tensor_tensor(
                    X[:, i + 1], pa, 1.0, X[:, i], op0=op.subtract, op1=op.mult)

            def prod(lo, hi, eng=None):
                n = hi - lo
                pr = prodp.tile([P, 5, 3, F], f16, tag="prod", name="prod")
                xb = X[:, lo:hi].unsqueeze(2).broadcast_to((P, n, 3, F))
                (eng or nc.vector).tensor_tensor(
                    pr[:, :n], dc[:, lo:hi], xb, op=op.mult)
                return pr

            def mms(pr, n, ka=0, stop_last=False):
                for j in range(n):
                    nc.tensor.matmul(
                        cacc[:], ident[:],
                        pr[:, j].rearrange("p c f -> p (c f)"),
                        start=False, stop=(stop_last and j == n - 1),
                        skip_group_check=True,
                    )
                if ka:
                    pe_keepalive(ka)

            def cnt_even(p):
                # cnt_{2p} = cnt_{2p-1} + q_{2p}
                nc.vector.tensor_tensor(
                    cntp[p][:, 1], cntp[p][:, 0], qB[:, 2 * p - 10], op=op.add)

            def cnt_odd(p):
                # cnt_{2p+1} = cnt_{2p-1} + (q_{2p} + q_{2p+1})
                nc.vector.tensor_tensor(
                    cntp[p + 1][:, 0], cntp[p][:, 0], qp[:, p - 5], op=op.add)

            # --- PE: c_0 accumulates straight from the dc tile (no product) ---
            nc.tensor.matmul(
                cacc[:], ident[:], dc[:, 0].rearrange("p c f -> p (c f)"),
                start=True, stop=False, skip_group_check=True,
            )
            pe_keepalive(4)

            # ---------------- DVE emission schedule ----------------
            # Phase A chain + count tree, laced so dependent chain steps
            # never run back-to-back (SBUF write-ack penalty).
            nc.vector.tensor_tensor(M04[:], am_even, am_odd, op=op.mult)
            nc.vector.tensor_tensor(s5[:], qA[:, 0:5], qA[:, 5:10], op=op.add)
            chainA(0)
            nc.vector.tensor_tensor(s2[:], s5[:, 0:2], s5[:, 2:4], op=op.add)
            chainA(1)
            nc.vector.tensor_tensor(s1[:], s2[:, 0], s2[:, 1], op=op.add)
            chainA(2)
            nc.vector.tensor_tensor(
                qp[:],
                qB[:, 0:8].rearrange("p (s two) f -> p s two f", two=2)[:, :, 0],
                qB[:, 0:8].rearrange("p (s two) f -> p s two f", two=2)[:, :, 1],
                op=op.add,
            )
            chainA(3)
            nc.vector.tensor_tensor(cntp[5][:, 0], s1[:], s5[:, 4], op=op.add)
            # odd X planes 1,3,5,7,9 in one strided op (needs X_0..X_8 even)
            nc.vector.tensor_tensor(X_oddA, X_evenA, am_even, op=op.mult)
            chainA(4)
            cnt_even(5)          # cnt_10
            gate(5)
            cnt_odd(5)           # cnt_11
            prA1 = prod(1, 6)
            mms(prA1, 5, ka=2)
            cnt_even(6)          # cnt_12
            gate(6)
            aeg(5)
            cnt_odd(6)           # cnt_13
            prA2 = prod(6, 11)
            mms(prA2, 5, ka=2)
            sttB(10)
            cnt_even(7)          # cnt_14
            gate(7)
            sttB(11)
            cnt_odd(7)           # cnt_15
            aeg(6)
            pr1112 = prod(11, 13)
            mms(pr1112, 2)
            sttB(12)
            cnt_even(8)          # cnt_16
            gate(8)
            sttB(13)
            cnt_odd(8)           # cnt_17
            aeg(7)
            pr1314 = prod(13, 15)
            mms(pr1314, 2)
            sttB(14)
            cnt_even(9)          # cnt_18
            gate(9)
            sttB(15)
            aeg(8)
            pr1516 = prod(15, 17)
            mms(pr1516, 2)
            sttB(16)
            sttB(17)
            aeg(9)
            pr1718 = prod(17, 19)
            mms(pr1718, 2)
            sttB(18)
            sttB(19)
            pr1920 = prod(19, 21)
            mms(pr1920, 2, stop_last=True)

            # tail: PSUM f32 -> SBUF f16 on ACT (DVE is already done)
            out_t = constp.tile([P, 3, F], f16, tag="out")
            nc.scalar.activation(
                out_t[:].rearrange("p c f -> p (c f)"), cacc[:],
                func=actf.Copy, bias=0.0, scale=1.0,
            )
            nc.sync.dma_start(out_d, out_t[:].rearrange("p c f -> p (c f)"))

    nc.compile()
    return nc


def _prep_inputs(color_stroke, alpha, depth):
    """Host prep: slice the top `depth` strokes (reversed: stroke 0 = highest
    index), resolve the alpha threshold in f32, and lay out per core in fp16:

      aeh [P, depth*F]:  planes 0-9  = am  = ae - 1   (ungated strokes)
                         planes 10-19 = ae             (gate applied on device)
      dc  [P, 21*3*F]:   dc_0 = c_0; dcS_i = (-1)^i (c_i - c_{i-1});
                         dc_20 = 1 - c_19   (white background fold)
    """
    a_r = alpha[:, N - depth:, 0][:, ::-1]               # (B, depth, P, F) f32
    ae0 = (a_r * (a_r > ALPHA_THRESH)).astype(np.float32)
    aeh = np.empty((B, depth, P, F), np.float16)
    aeh[:, :10] = (ae0[:, :10] - 1.0).astype(np.float16)
    aeh[:, 10:] = ae0[:, 10:].astype(np.float16)

    c_r = color_stroke[:, N - depth:][:, ::-1].astype(np.float32)  # (B,depth,3,P,F)
    dc = np.empty((B, depth + 1, 3, P, F), np.float32)
    dc[:, 0] = c_r[:, 0]
    dc[:, 1:depth] = c_r[:, 1:] - c_r[:, :-1]
    dc[:, depth] = 1.0 - c_r[:, depth - 1]
    dc[:, 1:depth:2] = -dc[:, 1:depth:2]                 # odd strokes negated
    dc16 = dc.astype(np.float16)

    ident = np.eye(P, dtype=np.float16)
    in_maps = []
    for b in range(B):
        a_core = np.ascontiguousarray(
            aeh[b].transpose(1, 0, 2)).reshape(P, depth * F)
        d_core = np.ascontiguousarray(
            dc16[b].transpose(2, 0, 1, 3)).reshape(P, (depth + 1) * 3 * F)
        in_maps.append({"aeh_in": a_core, "dc_in": d_core, "ident_in": ident})
    return in_maps


def _reference_numpy(color_stroke, alpha):
    """Exact replication of the oracle (incl. top-k tie-breaking) on host.
    Only used when the depth-cutoff precondition fails (pathological inputs)."""
    stroke_ids = np.arange(1, N + 1, dtype=np.int32).reshape(1, N, 1, 1)
    draw = stroke_ids * (alpha[:, :, 0] > ALPHA_THRESH).astype(np.int32)
    draw_t = np.moveaxis(draw, 1, -1)
    idx = np.argsort(-draw_t, axis=-1, kind="stable")[..., :K]
    idx = np.moveaxis(idx, -1, 1)[:, :, None]
    alpha_k = np.take_along_axis(alpha, idx, axis=1)
    color_k = np.take_along_axis(color_stroke, idx, axis=1)
    canvas = np.ones((B, 3, W, W), dtype=color_stroke.dtype)
    for i in range(K - 1, -1, -1):
        a = alpha_k[:, i]
        canvas = canvas * (1.0 - a) + a * color_k[:, i]
    return canvas


def kernel(color_stroke, alpha):
    color_stroke = np.asarray(color_stroke, dtype=np.float32)
    alpha = np.asarray(alpha, dtype=np.float32)
    assert color_stroke.shape == (B, N, 3, W, W), color_stroke.shape
    assert alpha.shape == (B, N, 1, W, W), alpha.shape

    # Precondition for the depth cutoff: every pixel finds its 10 passing
    # strokes within the top D.
    top_pass = (alpha[:, N - D:, 0] > ALPHA_THRESH).sum(axis=1)
    if top_pass.min() < K:
        return _reference_numpy(color_stroke, alpha)

    from concourse.bass_utils import run_bass_kernel_spmd

    if D not in _nc_cache:
        _nc_cache[D] = _build_nc(D)
    nc = _nc_cache[D]

    in_maps = _prep_inputs(color_stroke, alpha, D)
    res = run_bass_kernel_spmd(nc, in_maps, core_ids=list(range(NCORES)))

    out = np.empty((B, 3, W, W), dtype=np.float32)
    for b in range(B):
        out[b] = (
            res.results[b]["out"].astype(np.float32).reshape(P, 3, F).transpose(1, 0, 2)
        )
    return out
